# revision 2
# baseline (speedup 1.0000x reference)
"""DistanceAutoMLCriterion loss on 8 Trainium2 NeuronCores (Bass/Tile) — v3.

Structure (per core: pred [1024, 32000] f32, 8 groups of 128 tokens):
  - stream 40 DMAs of 3.27 MB ([128, 16, 400] f32) alternating between the
    two HWDGE rings (sync/scalar), XBUFS=5 deep; DVE reduce_max per tile
    into the persistent M1 [P, G, 80] (sub-chunk maxima, S=400).
  - token-indexed gathers (weight[target], alpha/beta, pred_ll[target]) and
    gold-norms (ScalarE) issued up front, overlapping the stream.
  - per-group argmax refinement software-pipelined at group granularity:
      phase A(g): row max m_g, winning sub-chunk via descending-iota max
                  trick, issue sub-chunk re-fetch gather
      phase B(g): argmax column inside the 400-wide sub-chunk, issue
                  weight[argmax] gather            (emitted after A(g+1))
      phase C(g): dot(gold,pe) on DVE + ||pe||^2 on ScalarE
                  (emitted after B(g+1))
    so indirect-DMA latencies never head-block the DVE queue.
  - one batched [P,G] finish: cosine, sigmoid, loss, partition-sum matmul.
Host adds the 8 partial scalar sums.
"""

import numpy as np

import concourse.bacc as bacc
import concourse.bass as bass
import concourse.bass_utils as bass_utils
import concourse.tile as tile
from concourse import mybir
from concourse.bass import IndirectOffsetOnAxis

P = 128
N, V, D = 8192, 32000, 512
CORES = 8
TOK = N // CORES          # 1024 tokens per core
G = TOK // P              # 8 groups per core
S = 400                   # sub-chunk width for argmax refinement
NS = V // S               # 80 sub-chunks per row
CSUB = 16                 # sub-chunks per streamed tile
CV = CSUB * S             # 6400 f32 per streamed tile (3.27 MB per DMA)
NK = NS // CSUB           # 5 streamed tiles per group
BIG = 8388608.0           # 2^23 — exact in f32, > any index used below
EPS = 1e-8

f32 = mybir.dt.float32
i32 = mybir.dt.int32

XBUFS = 5                 # stream-tile buffers (25600 B/partition each)
ALT_ENGINE = True         # alternate big loads between sync/scalar HWDGE rings

_CACHE = {}


def _build(reps=1, variant="full"):
    """variant: 'full' | 'dma' (stream loads only) | 'reduce' | 'dve'."""
    key = (reps, variant, XBUFS, ALT_ENGINE, CSUB, S)
    if key in _CACHE:
        return _CACHE[key]

    nc = bacc.Bacc("TRN2", target_bir_lowering=False, debug=False)

    pred = nc.dram_tensor("pred", [TOK, V], f32, kind="ExternalInput")
    tgt = nc.dram_tensor("tgt", [P, G], i32, kind="ExternalInput")
    # tok*V + tgt, host-computed: int32 adds above 2^24 are not exact on DVE
    fidx_in = nc.dram_tensor("fidx", [P, G], i32, kind="ExternalInput")
    rbase = nc.dram_tensor("rbase", [P, G], i32, kind="ExternalInput")  # tok*NS
    wt = nc.dram_tensor("wt", [V, D], f32, kind="ExternalInput")
    ab = nc.dram_tensor("ab", [V, 2], f32, kind="ExternalInput")
    outs = [
        nc.dram_tensor("out" if r == 0 else f"out{r}", [1, 2], f32,
                       kind="ExternalOutput")
        for r in range(reps)
    ]

    pred2d = pred[:, :]
    pred_el = pred2d.rearrange("n (v w) -> (n v) w", w=1)     # [TOK*V, 1]
    pred_sub = pred2d.rearrange("n (s w) -> (n s) w", w=S)    # [TOK*NS, S]
    pred3 = pred2d.rearrange("n (k c s) -> n k c s", c=CSUB, s=S)

    with tile.TileContext(nc) as tc:
        with (
            tc.tile_pool(name="xpool", bufs=XBUFS) as xpool,
            tc.tile_pool(name="per", bufs=1) as per,      # persistent, 1 buf/tag
            tc.tile_pool(name="subp", bufs=2) as subp,    # per-group sub-chunks
            tc.tile_pool(name="psum", bufs=1, space="PSUM") as psum,
        ):
            # ---- one-time setup (small loads on gpsimd: keep rings clear) --
            tgt_sb = per.tile([P, G], i32)
            fidx = per.tile([P, G], i32)
            rbase_sb = per.tile([P, G], i32)
            nc.gpsimd.dma_start(out=tgt_sb[:, :], in_=tgt[:, :])
            nc.gpsimd.dma_start(out=fidx[:, :], in_=fidx_in[:, :])
            nc.gpsimd.dma_start(out=rbase_sb[:, :], in_=rbase[:, :])

            # -mask:  -(target != 0)
            mn = per.tile([P, G], f32)
            nc.vector.tensor_scalar(
                mn[:, :], tgt_sb[:, :], 0.0, -1.0,
                op0=mybir.AluOpType.not_equal, op1=mybir.AluOpType.mult,
            )

            # descending iotas: idesc[s] = BIG - s (exact in f32)
            iota_ns_i = per.tile([P, NS], i32)
            nc.gpsimd.iota(iota_ns_i[:, :], pattern=[[1, NS]],
                           base=0, channel_multiplier=0)
            idesc_ns = per.tile([P, NS], f32)
            nc.vector.tensor_scalar(
                idesc_ns[:, :], iota_ns_i[:, :], -1.0, BIG,
                op0=mybir.AluOpType.mult, op1=mybir.AluOpType.add,
            )
            iota_s_i = per.tile([P, S], i32)
            nc.gpsimd.iota(iota_s_i[:, :], pattern=[[1, S]],
                           base=0, channel_multiplier=0)
            idesc_s = per.tile([P, S], f32)
            nc.vector.tensor_scalar(
                idesc_s[:, :], iota_s_i[:, :], -1.0, BIG,
                op0=mybir.AluOpType.mult, op1=mybir.AluOpType.add,
            )

            ones = per.tile([P, 1], f32)
            nc.vector.memset(ones[:, :], 1.0)

            if variant == "dve":
                xt0 = per.tile([P, CSUB, S], f32, name="xtd", tag="xtd")
                nc.sync.dma_start(out=xt0[:, :, :], in_=pred3[0:P, 0, :, :])

            # reps>1 duplicates the whole computation for slope-based timing
            for rep in range(reps):
              out = outs[rep]

              if variant != "full":
                M1v = per.tile([P, NS], f32, name=f"M1v{rep}", tag="M1v")
                if variant == "dve":
                    for j in range(G * NK):
                        nc.vector.reduce_max(
                            out=M1v[:, (j % NK) * CSUB : (j % NK + 1) * CSUB],
                            in_=xt0[:, :, :],
                            axis=mybir.AxisListType.X,
                        )
                else:
                    for g in range(G):
                        gsl = slice(g * P, (g + 1) * P)
                        for k in range(NK):
                            xt = xpool.tile([P, CSUB, S], f32, name=f"xtv{rep}",
                                            tag="xt")
                            eng = (nc.scalar
                                   if ALT_ENGINE and (g * NK + k) % 2
                                   else nc.sync)
                            eng.dma_start(out=xt[:, :, :],
                                          in_=pred3[gsl, k, :, :])
                            if variant == "reduce":
                                nc.vector.reduce_max(
                                    out=M1v[:, k * CSUB : (k + 1) * CSUB],
                                    in_=xt[:, :, :],
                                    axis=mybir.AxisListType.X,
                                )
                resv = per.tile([1, 2], f32, name=f"resv{rep}", tag="res")
                nc.vector.memset(resv[:, :], 0.0)
                nc.sync.dma_start(out=out[:, :], in_=resv[:, :])
                continue

              # ---- early gathers (overlap the stream) --------------------
              t_val = per.tile([P, G], f32, name=f"t_val{rep}", tag="t_val")
              for g in range(G):
                  nc.gpsimd.indirect_dma_start(
                      out=t_val[:, g : g + 1], out_offset=None,
                      in_=pred_el,
                      in_offset=IndirectOffsetOnAxis(ap=fidx[:, g : g + 1],
                                                     axis=0),
                  )
              gold = per.tile([P, G, D], f32, name=f"gold{rep}", tag="gold")
              for g in range(G):
                  nc.gpsimd.indirect_dma_start(
                      out=gold[:, g, :], out_offset=None,
                      in_=wt[:, :],
                      in_offset=IndirectOffsetOnAxis(ap=tgt_sb[:, g : g + 1],
                                                     axis=0),
                  )
              abg = per.tile([P, G, 2], f32, name=f"abg{rep}", tag="abg")
              for g in range(G):
                  nc.gpsimd.indirect_dma_start(
                      out=abg[:, g, :], out_offset=None,
                      in_=ab[:, :],
                      in_offset=IndirectOffsetOnAxis(ap=tgt_sb[:, g : g + 1],
                                                     axis=0),
                  )
              # ||gold||^2 per (p,g) on ScalarE (overlaps stream + DVE)
              nanb2 = per.tile([P, 2 * G], f32, name=f"nanb2{rep}", tag="nanb2")
              sqd = per.tile([P, D], f32, name=f"sqd{rep}", tag="sqd")
              for g in range(G):
                  nc.scalar.activation(
                      sqd[:, :], gold[:, g, :],
                      mybir.ActivationFunctionType.Square,
                      accum_out=nanb2[:, g : g + 1],
                  )

              # ---- per-rep persistent tiles ------------------------------
              M1 = per.tile([P, G, NS], f32, name=f"M1_{rep}", tag="M1")
              m_all = per.tile([P, G], f32, name=f"m_all{rep}", tag="m_all")
              w1 = per.tile([P, G], f32, name=f"w1_{rep}", tag="w1")
              veq = per.tile([P, NS], f32, name=f"veq{rep}", tag="veq")
              veq2 = per.tile([P, S], f32, name=f"veq2_{rep}", tag="veq2")
              w2 = per.tile([P, G], f32, name=f"w2_{rep}", tag="w2")
              jf = per.tile([P, G], f32, name=f"jf{rep}", tag="jf")
              sf = per.tile([P, G], f32, name=f"sf{rep}", tag="sf")
              pos = per.tile([P, G], i32, name=f"pos{rep}", tag="pos")
              s_i = per.tile([P, G], i32, name=f"s_i{rep}", tag="s_i")
              ridx = per.tile([P, G], i32, name=f"ridx{rep}", tag="ridx")
              pe = per.tile([P, G, D], f32, name=f"pe{rep}", tag="pe")
              dot = per.tile([P, G], f32, name=f"dot{rep}", tag="dot")
              dotd = per.tile([P, D], f32, name=f"dotd{rep}", tag="dotd")
              sqd2 = per.tile([P, D], f32, name=f"sqd2_{rep}", tag="sqd2")
              subs = {}

              def phase_a(g):
                  # row max of group g + winning sub-chunk; issue re-fetch
                  nc.vector.reduce_max(out=m_all[:, g : g + 1],
                                       in_=M1[:, g, :],
                                       axis=mybir.AxisListType.X)
                  # veq = (M1_g == m_g) * (BIG - s): max -> BIG - s*
                  nc.vector.scalar_tensor_tensor(
                      out=veq[:, :], in0=M1[:, g, :],
                      scalar=m_all[:, g : g + 1], in1=idesc_ns[:, :],
                      op0=mybir.AluOpType.is_equal, op1=mybir.AluOpType.mult,
                  )
                  nc.vector.reduce_max(out=w1[:, g : g + 1], in_=veq[:, :],
                                       axis=mybir.AxisListType.X)
                  nc.vector.tensor_scalar(
                      s_i[:, g : g + 1], w1[:, g : g + 1], -1.0, BIG,
                      op0=mybir.AluOpType.mult, op1=mybir.AluOpType.add,
                  )
                  nc.vector.tensor_add(ridx[:, g : g + 1],
                                       rbase_sb[:, g : g + 1],
                                       s_i[:, g : g + 1])
                  sub = subp.tile([P, S], f32, name=f"sub{rep}_{g}", tag="sub")
                  subs[g] = sub
                  nc.gpsimd.indirect_dma_start(
                      out=sub[:, :], out_offset=None,
                      in_=pred_sub,
                      in_offset=IndirectOffsetOnAxis(ap=ridx[:, g : g + 1],
                                                     axis=0),
                  )

              def phase_b(g):
                  # argmax column inside the re-fetched sub-chunk; issue
                  # weight[argmax] gather
                  sub = subs.pop(g)
                  nc.vector.scalar_tensor_tensor(
                      out=veq2[:, :], in0=sub[:, :],
                      scalar=m_all[:, g : g + 1], in1=idesc_s[:, :],
                      op0=mybir.AluOpType.is_equal, op1=mybir.AluOpType.mult,
                  )
                  nc.vector.reduce_max(out=w2[:, g : g + 1], in_=veq2[:, :],
                                       axis=mybir.AxisListType.X)
                  # position = s* * S + j, all in small exact integers:
                  # jf = BIG - w2 (= j), sf = BIG - w1 (= s*), pos = sf*S + jf.
                  # (S*w1 would be ~2^31.6 where f32 ulp=256 — inexact for
                  # non-power-of-2 S.)
                  nc.vector.tensor_scalar(
                      jf[:, g : g + 1], w2[:, g : g + 1], -1.0, BIG,
                      op0=mybir.AluOpType.mult, op1=mybir.AluOpType.add,
                  )
                  nc.vector.tensor_scalar(
                      sf[:, g : g + 1], w1[:, g : g + 1], -1.0, BIG,
                      op0=mybir.AluOpType.mult, op1=mybir.AluOpType.add,
                  )
                  nc.vector.scalar_tensor_tensor(
                      out=pos[:, g : g + 1], in0=sf[:, g : g + 1],
                      scalar=float(S), in1=jf[:, g : g + 1],
                      op0=mybir.AluOpType.mult, op1=mybir.AluOpType.add,
                  )
                  nc.gpsimd.indirect_dma_start(
                      out=pe[:, g, :], out_offset=None,
                      in_=wt[:, :],
                      in_offset=IndirectOffsetOnAxis(ap=pos[:, g : g + 1],
                                                     axis=0),
                  )

              def phase_c(g):
                  # dot(gold, pe) on DVE; ||pe||^2 on ScalarE
                  nc.vector.scalar_tensor_tensor(
                      out=dotd[:, :], in0=gold[:, g, :], scalar=0.0,
                      in1=pe[:, g, :],
                      op0=mybir.AluOpType.bypass, op1=mybir.AluOpType.mult,
                      accum_out=dot[:, g : g + 1],
                  )
                  nc.scalar.activation(
                      sqd2[:, :], pe[:, g, :],
                      mybir.ActivationFunctionType.Square,
                      accum_out=nanb2[:, G + g : G + g + 1],
                  )

              # ---- stream + pipelined per-group refinement ---------------
              for g in range(G):
                gsl = slice(g * P, (g + 1) * P)
                for k in range(NK):
                    xt = xpool.tile([P, CSUB, S], f32, name=f"xt{rep}", tag="xt")
                    eng = (nc.scalar if ALT_ENGINE and (g * NK + k) % 2 else
                           nc.sync)
                    eng.dma_start(out=xt[:, :, :], in_=pred3[gsl, k, :, :])
                    nc.vector.reduce_max(
                        out=M1[:, g, k * CSUB : (k + 1) * CSUB],
                        in_=xt[:, :, :],
                        axis=mybir.AxisListType.X,
                    )
                phase_a(g)
                if g >= 1:
                    phase_b(g - 1)
                if g >= 2:
                    phase_c(g - 2)

              phase_b(G - 1)
              phase_c(G - 2)
              phase_c(G - 1)

              # ---- batched finish ----------------------------------------
              # cosine: dist = dot / (max(sqrt(na2),eps)*max(sqrt(nb2),eps))
              nanb = per.tile([P, 2 * G], f32, name=f"nanb{rep}", tag="nanb")
              nc.scalar.activation(nanb[:, :], nanb2[:, :],
                                   mybir.ActivationFunctionType.Sqrt)
              nc.vector.tensor_scalar_max(nanb[:, :], nanb[:, :], EPS)
              den = per.tile([P, G], f32, name=f"den{rep}", tag="den")
              nc.vector.tensor_mul(den[:, :], nanb[:, 0:G], nanb[:, G : 2 * G])
              rden = per.tile([P, G], f32, name=f"rden{rep}", tag="rden")
              nc.vector.reciprocal(rden[:, :], den[:, :])
              dist = per.tile([P, G], f32, name=f"dist{rep}", tag="dist")
              nc.vector.tensor_mul(dist[:, :], dot[:, :], rden[:, :])

              # sig = sigmoid(alpha*dist + beta)
              sarg = per.tile([P, G], f32, name=f"sarg{rep}", tag="sarg")
              nc.vector.tensor_mul(sarg[:, :], dist[:, :], abg[:, :, 0])
              nc.vector.tensor_add(sarg[:, :], sarg[:, :], abg[:, :, 1])
              sig = per.tile([P, G], f32, name=f"sig{rep}", tag="sig")
              nc.scalar.activation(sig[:, :], sarg[:, :],
                                   mybir.ActivationFunctionType.Sigmoid)

              # loss*mask = (sig/2)*(nllm - plm) + 0.5*(nllm + plm)
              #   nllm = nll*mask = t_val*mn;  plm = -m*mask = m_all*mn
              nllm = per.tile([P, G], f32, name=f"nllm{rep}", tag="nllm")
              nc.vector.tensor_mul(nllm[:, :], t_val[:, :], mn[:, :])
              plm = per.tile([P, G], f32, name=f"plm{rep}", tag="plm")
              nc.vector.tensor_mul(plm[:, :], m_all[:, :], mn[:, :])
              d1 = per.tile([P, G], f32, name=f"d1_{rep}", tag="d1")
              nc.vector.tensor_sub(d1[:, :], nllm[:, :], plm[:, :])
              d2 = per.tile([P, G], f32, name=f"d2_{rep}", tag="d2")
              nc.vector.tensor_add(d2[:, :], nllm[:, :], plm[:, :])
              xd = per.tile([P, G], f32, name=f"xd{rep}", tag="xd")
              nc.vector.tensor_mul(xd[:, :], sig[:, :], d1[:, :])
              lt = per.tile([P, G], f32, name=f"lt{rep}", tag="lt")
              nc.vector.tensor_add(lt[:, :], xd[:, :], d2[:, :])
              loss_t = per.tile([P, G], f32, name=f"loss_t{rep}", tag="loss_t")
              nc.vector.tensor_scalar_mul(loss_t[:, :], lt[:, :], 0.5)

              vals = per.tile([P, 2], f32, name=f"vals{rep}", tag="vals")
              nc.vector.reduce_sum(out=vals[:, 0:1], in_=loss_t[:, :],
                                   axis=mybir.AxisListType.X)
              nc.vector.reduce_sum(out=vals[:, 1:2], in_=nllm[:, :],
                                   axis=mybir.AxisListType.X)
              acc = psum.tile([1, 2], f32, space="PSUM", name=f"acc{rep}",
                              tag="acc")
              nc.tensor.matmul(out=acc[:, :], lhsT=ones[:, :], rhs=vals[:, :],
                               start=True, stop=True)
              res = per.tile([1, 2], f32, name=f"res{rep}", tag="res2")
              nc.vector.tensor_copy(res[:, :], acc[:, :])
              nc.sync.dma_start(out=out[:, :], in_=res[:, :])

    nc.compile()
    _CACHE[key] = nc
    return nc


def _host_constants():
    toks = np.arange(TOK, dtype=np.int64)
    rbase = (toks * NS).astype(np.int32).reshape(G, P).T.copy()
    return rbase


def _in_maps(pred_ll, target, weight, alpha, beta):
    rbase = _host_constants()
    pred_ll = np.ascontiguousarray(pred_ll, dtype=np.float32)
    weight = np.ascontiguousarray(weight, dtype=np.float32)
    ab = np.ascontiguousarray(
        np.stack([np.asarray(alpha, np.float32), np.asarray(beta, np.float32)],
                 axis=1)
    )
    tgt64 = np.asarray(target).astype(np.int64)
    toks = np.arange(TOK, dtype=np.int64)

    in_maps = []
    for c in range(CORES):
        tl = tgt64[c * TOK : (c + 1) * TOK]
        fidx = (toks * V + tl).astype(np.int32)
        in_maps.append({
            "pred": pred_ll[c * TOK : (c + 1) * TOK],
            "tgt": np.ascontiguousarray(tl.astype(np.int32).reshape(G, P).T),
            "fidx": np.ascontiguousarray(fidx.reshape(G, P).T),
            "rbase": rbase,
            "wt": weight,
            "ab": ab,
        })
    return in_maps


def _finish(results):
    partial = np.stack([r["out"].reshape(2) for r in results])  # [8, 2]
    loss_sum, nll_sum = np.asarray(partial, np.float64).sum(axis=0)
    return (np.float32(loss_sum), np.float32(nll_sum))


def kernel(pred_ll, target, weight, alpha, beta):
    nc = _build()
    in_maps = _in_maps(pred_ll, target, weight, alpha, beta)
    res = bass_utils.run_bass_kernel_spmd(nc, in_maps, core_ids=list(range(CORES)))
    return _finish(res.results)


# revision 3
# speedup vs baseline: 1.0172x; 1.0172x over previous
"""DistanceAutoMLCriterion loss on 8 Trainium2 NeuronCores (Bass/Tile) — v3.

Structure (per core: pred [1024, 32000] f32, 8 groups of 128 tokens):
  - stream 40 DMAs of 3.27 MB ([128, 16, 400] f32) alternating between the
    two HWDGE rings (sync/scalar), XBUFS=5 deep; DVE reduce_max per tile
    into the persistent M1 [P, G, 80] (sub-chunk maxima, S=400).
  - token-indexed gathers (weight[target], alpha/beta, pred_ll[target]) and
    gold-norms (ScalarE) issued up front, overlapping the stream.
  - per-group argmax refinement software-pipelined at group granularity:
      phase A(g): row max m_g, winning sub-chunk via descending-iota max
                  trick, issue sub-chunk re-fetch gather
      phase B(g): argmax column inside the 400-wide sub-chunk, issue
                  weight[argmax] gather            (emitted after A(g+1))
      phase C(g): dot(gold,pe) on DVE + ||pe||^2 on ScalarE
                  (emitted after B(g+1))
    so indirect-DMA latencies never head-block the DVE queue.
  - one batched [P,G] finish: cosine, sigmoid, loss, partition-sum matmul.
Host adds the 8 partial scalar sums.
"""

import numpy as np

import concourse.bacc as bacc
import concourse.bass as bass
import concourse.bass_utils as bass_utils
import concourse.tile as tile
from concourse import mybir
from concourse.bass import IndirectOffsetOnAxis

P = 128
N, V, D = 8192, 32000, 512
CORES = 8
TOK = N // CORES          # 1024 tokens per core
G = TOK // P              # 8 groups per core
S = 400                   # sub-chunk width for argmax refinement
NS = V // S               # 80 sub-chunks per row
CSUB = 40                 # sub-chunks per streamed tile
CV = CSUB * S             # 16000 f32 per streamed tile (8.19 MB per DMA)
NK = NS // CSUB           # 2 streamed tiles per group
BIG = 8388608.0           # 2^23 — exact in f32, > any index used below
EPS = 1e-8

f32 = mybir.dt.float32
i32 = mybir.dt.int32

XBUFS = 2                 # stream-tile buffers (64000 B/partition each)
ALT_ENGINE = True         # alternate big loads between sync/scalar HWDGE rings

_CACHE = {}


def _build(reps=1, variant="full"):
    """variant: 'full' | 'dma' (stream loads only) | 'reduce' | 'dve'."""
    key = (reps, variant, XBUFS, ALT_ENGINE, CSUB, S)
    if key in _CACHE:
        return _CACHE[key]

    nc = bacc.Bacc("TRN2", target_bir_lowering=False, debug=False)

    pred = nc.dram_tensor("pred", [TOK, V], f32, kind="ExternalInput")
    tgt = nc.dram_tensor("tgt", [P, G], i32, kind="ExternalInput")
    # tok*V + tgt, host-computed: int32 adds above 2^24 are not exact on DVE
    fidx_in = nc.dram_tensor("fidx", [P, G], i32, kind="ExternalInput")
    rbase = nc.dram_tensor("rbase", [P, G], i32, kind="ExternalInput")  # tok*NS
    wt = nc.dram_tensor("wt", [V, D], f32, kind="ExternalInput")
    ab = nc.dram_tensor("ab", [V, 2], f32, kind="ExternalInput")
    outs = [
        nc.dram_tensor("out" if r == 0 else f"out{r}", [1, 2], f32,
                       kind="ExternalOutput")
        for r in range(reps)
    ]

    pred2d = pred[:, :]
    pred_el = pred2d.rearrange("n (v w) -> (n v) w", w=1)     # [TOK*V, 1]
    pred_sub = pred2d.rearrange("n (s w) -> (n s) w", w=S)    # [TOK*NS, S]
    pred3 = pred2d.rearrange("n (k c s) -> n k c s", c=CSUB, s=S)

    with tile.TileContext(nc) as tc:
        with (
            tc.tile_pool(name="xpool", bufs=XBUFS) as xpool,
            tc.tile_pool(name="per", bufs=1) as per,      # persistent, 1 buf/tag
            tc.tile_pool(name="subp", bufs=2) as subp,    # per-group sub-chunks
            tc.tile_pool(name="psum", bufs=1, space="PSUM") as psum,
        ):
            # ---- one-time setup (small loads on gpsimd: keep rings clear) --
            tgt_sb = per.tile([P, G], i32)
            fidx = per.tile([P, G], i32)
            rbase_sb = per.tile([P, G], i32)
            nc.gpsimd.dma_start(out=tgt_sb[:, :], in_=tgt[:, :])
            nc.gpsimd.dma_start(out=fidx[:, :], in_=fidx_in[:, :])
            nc.gpsimd.dma_start(out=rbase_sb[:, :], in_=rbase[:, :])

            # -mask:  -(target != 0)
            mn = per.tile([P, G], f32)
            nc.vector.tensor_scalar(
                mn[:, :], tgt_sb[:, :], 0.0, -1.0,
                op0=mybir.AluOpType.not_equal, op1=mybir.AluOpType.mult,
            )

            # descending iotas: idesc[s] = BIG - s (exact in f32)
            iota_ns_i = per.tile([P, NS], i32)
            nc.gpsimd.iota(iota_ns_i[:, :], pattern=[[1, NS]],
                           base=0, channel_multiplier=0)
            idesc_ns = per.tile([P, NS], f32)
            nc.vector.tensor_scalar(
                idesc_ns[:, :], iota_ns_i[:, :], -1.0, BIG,
                op0=mybir.AluOpType.mult, op1=mybir.AluOpType.add,
            )
            iota_s_i = per.tile([P, S], i32)
            nc.gpsimd.iota(iota_s_i[:, :], pattern=[[1, S]],
                           base=0, channel_multiplier=0)
            idesc_s = per.tile([P, S], f32)
            nc.vector.tensor_scalar(
                idesc_s[:, :], iota_s_i[:, :], -1.0, BIG,
                op0=mybir.AluOpType.mult, op1=mybir.AluOpType.add,
            )

            ones = per.tile([P, 1], f32)
            nc.vector.memset(ones[:, :], 1.0)

            if variant == "dve":
                xt0 = per.tile([P, CSUB, S], f32, name="xtd", tag="xtd")
                nc.sync.dma_start(out=xt0[:, :, :], in_=pred3[0:P, 0, :, :])

            # reps>1 duplicates the whole computation for slope-based timing
            for rep in range(reps):
              out = outs[rep]

              if variant != "full":
                M1v = per.tile([P, NS], f32, name=f"M1v{rep}", tag="M1v")
                if variant == "dve":
                    for j in range(G * NK):
                        nc.vector.reduce_max(
                            out=M1v[:, (j % NK) * CSUB : (j % NK + 1) * CSUB],
                            in_=xt0[:, :, :],
                            axis=mybir.AxisListType.X,
                        )
                else:
                    for g in range(G):
                        gsl = slice(g * P, (g + 1) * P)
                        for k in range(NK):
                            xt = xpool.tile([P, CSUB, S], f32, name=f"xtv{rep}",
                                            tag="xt")
                            eng = (nc.scalar
                                   if ALT_ENGINE and (g * NK + k) % 2
                                   else nc.sync)
                            eng.dma_start(out=xt[:, :, :],
                                          in_=pred3[gsl, k, :, :])
                            if variant == "reduce":
                                nc.vector.reduce_max(
                                    out=M1v[:, k * CSUB : (k + 1) * CSUB],
                                    in_=xt[:, :, :],
                                    axis=mybir.AxisListType.X,
                                )
                resv = per.tile([1, 2], f32, name=f"resv{rep}", tag="res")
                nc.vector.memset(resv[:, :], 0.0)
                nc.sync.dma_start(out=out[:, :], in_=resv[:, :])
                continue

              # ---- early gathers (overlap the stream) --------------------
              t_val = per.tile([P, G], f32, name=f"t_val{rep}", tag="t_val")
              for g in range(G):
                  nc.gpsimd.indirect_dma_start(
                      out=t_val[:, g : g + 1], out_offset=None,
                      in_=pred_el,
                      in_offset=IndirectOffsetOnAxis(ap=fidx[:, g : g + 1],
                                                     axis=0),
                  )
              gold = per.tile([P, G, D], f32, name=f"gold{rep}", tag="gold")
              for g in range(G):
                  nc.gpsimd.indirect_dma_start(
                      out=gold[:, g, :], out_offset=None,
                      in_=wt[:, :],
                      in_offset=IndirectOffsetOnAxis(ap=tgt_sb[:, g : g + 1],
                                                     axis=0),
                  )
              abg = per.tile([P, G, 2], f32, name=f"abg{rep}", tag="abg")
              for g in range(G):
                  nc.gpsimd.indirect_dma_start(
                      out=abg[:, g, :], out_offset=None,
                      in_=ab[:, :],
                      in_offset=IndirectOffsetOnAxis(ap=tgt_sb[:, g : g + 1],
                                                     axis=0),
                  )
              # ||gold||^2 per (p,g) on ScalarE (overlaps stream + DVE)
              nanb2 = per.tile([P, 2 * G], f32, name=f"nanb2{rep}", tag="nanb2")
              sqd = per.tile([P, D], f32, name=f"sqd{rep}", tag="sqd")
              for g in range(G):
                  nc.scalar.activation(
                      sqd[:, :], gold[:, g, :],
                      mybir.ActivationFunctionType.Square,
                      accum_out=nanb2[:, g : g + 1],
                  )

              # ---- per-rep persistent tiles ------------------------------
              M1 = per.tile([P, G, NS], f32, name=f"M1_{rep}", tag="M1")
              m_all = per.tile([P, G], f32, name=f"m_all{rep}", tag="m_all")
              w1 = per.tile([P, G], f32, name=f"w1_{rep}", tag="w1")
              veq = per.tile([P, NS], f32, name=f"veq{rep}", tag="veq")
              veq2 = per.tile([P, S], f32, name=f"veq2_{rep}", tag="veq2")
              w2 = per.tile([P, G], f32, name=f"w2_{rep}", tag="w2")
              jf = per.tile([P, G], f32, name=f"jf{rep}", tag="jf")
              sf = per.tile([P, G], f32, name=f"sf{rep}", tag="sf")
              pos = per.tile([P, G], i32, name=f"pos{rep}", tag="pos")
              s_i = per.tile([P, G], i32, name=f"s_i{rep}", tag="s_i")
              ridx = per.tile([P, G], i32, name=f"ridx{rep}", tag="ridx")
              pe = per.tile([P, G, D], f32, name=f"pe{rep}", tag="pe")
              dot = per.tile([P, G], f32, name=f"dot{rep}", tag="dot")
              dotd = per.tile([P, D], f32, name=f"dotd{rep}", tag="dotd")
              sqd2 = per.tile([P, D], f32, name=f"sqd2_{rep}", tag="sqd2")
              subs = {}

              def phase_a(g):
                  # row max of group g + winning sub-chunk; issue re-fetch
                  nc.vector.reduce_max(out=m_all[:, g : g + 1],
                                       in_=M1[:, g, :],
                                       axis=mybir.AxisListType.X)
                  # veq = (M1_g == m_g) * (BIG - s): max -> BIG - s*
                  nc.vector.scalar_tensor_tensor(
                      out=veq[:, :], in0=M1[:, g, :],
                      scalar=m_all[:, g : g + 1], in1=idesc_ns[:, :],
                      op0=mybir.AluOpType.is_equal, op1=mybir.AluOpType.mult,
                  )
                  nc.vector.reduce_max(out=w1[:, g : g + 1], in_=veq[:, :],
                                       axis=mybir.AxisListType.X)
                  nc.vector.tensor_scalar(
                      s_i[:, g : g + 1], w1[:, g : g + 1], -1.0, BIG,
                      op0=mybir.AluOpType.mult, op1=mybir.AluOpType.add,
                  )
                  nc.vector.tensor_add(ridx[:, g : g + 1],
                                       rbase_sb[:, g : g + 1],
                                       s_i[:, g : g + 1])
                  sub = subp.tile([P, S], f32, name=f"sub{rep}_{g}", tag="sub")
                  subs[g] = sub
                  nc.gpsimd.indirect_dma_start(
                      out=sub[:, :], out_offset=None,
                      in_=pred_sub,
                      in_offset=IndirectOffsetOnAxis(ap=ridx[:, g : g + 1],
                                                     axis=0),
                  )

              def phase_b(g):
                  # argmax column inside the re-fetched sub-chunk; issue
                  # weight[argmax] gather
                  sub = subs.pop(g)
                  nc.vector.scalar_tensor_tensor(
                      out=veq2[:, :], in0=sub[:, :],
                      scalar=m_all[:, g : g + 1], in1=idesc_s[:, :],
                      op0=mybir.AluOpType.is_equal, op1=mybir.AluOpType.mult,
                  )
                  nc.vector.reduce_max(out=w2[:, g : g + 1], in_=veq2[:, :],
                                       axis=mybir.AxisListType.X)
                  # position = s* * S + j, all in small exact integers:
                  # jf = BIG - w2 (= j), sf = BIG - w1 (= s*), pos = sf*S + jf.
                  # (S*w1 would be ~2^31.6 where f32 ulp=256 — inexact for
                  # non-power-of-2 S.)
                  nc.vector.tensor_scalar(
                      jf[:, g : g + 1], w2[:, g : g + 1], -1.0, BIG,
                      op0=mybir.AluOpType.mult, op1=mybir.AluOpType.add,
                  )
                  nc.vector.tensor_scalar(
                      sf[:, g : g + 1], w1[:, g : g + 1], -1.0, BIG,
                      op0=mybir.AluOpType.mult, op1=mybir.AluOpType.add,
                  )
                  nc.vector.scalar_tensor_tensor(
                      out=pos[:, g : g + 1], in0=sf[:, g : g + 1],
                      scalar=float(S), in1=jf[:, g : g + 1],
                      op0=mybir.AluOpType.mult, op1=mybir.AluOpType.add,
                  )
                  nc.gpsimd.indirect_dma_start(
                      out=pe[:, g, :], out_offset=None,
                      in_=wt[:, :],
                      in_offset=IndirectOffsetOnAxis(ap=pos[:, g : g + 1],
                                                     axis=0),
                  )

              def phase_c(g):
                  # dot(gold, pe) on DVE; ||pe||^2 on ScalarE
                  nc.vector.scalar_tensor_tensor(
                      out=dotd[:, :], in0=gold[:, g, :], scalar=0.0,
                      in1=pe[:, g, :],
                      op0=mybir.AluOpType.bypass, op1=mybir.AluOpType.mult,
                      accum_out=dot[:, g : g + 1],
                  )
                  nc.scalar.activation(
                      sqd2[:, :], pe[:, g, :],
                      mybir.ActivationFunctionType.Square,
                      accum_out=nanb2[:, G + g : G + g + 1],
                  )

              # ---- stream + pipelined per-group refinement ---------------
              for g in range(G):
                gsl = slice(g * P, (g + 1) * P)
                for k in range(NK):
                    eng = (nc.scalar if ALT_ENGINE and (g * NK + k) % 2 else
                           nc.sync)
                    if g == G - 1 and k == NK - 1:
                        # split the final tile so the end-of-stream reduce
                        # drain is half as long
                        h = CSUB // 2
                        for half in range(2):
                            xt = xpool.tile([P, CSUB, S], f32,
                                            name=f"xt{rep}", tag="xt")
                            e2 = nc.sync if half == 0 else nc.scalar
                            e2.dma_start(
                                out=xt[:, 0:h, :],
                                in_=pred3[gsl, k, half * h : (half + 1) * h, :],
                            )
                            nc.vector.reduce_max(
                                out=M1[:, g, k * CSUB + half * h :
                                       k * CSUB + (half + 1) * h],
                                in_=xt[:, 0:h, :],
                                axis=mybir.AxisListType.X,
                            )
                        continue
                    xt = xpool.tile([P, CSUB, S], f32, name=f"xt{rep}", tag="xt")
                    eng.dma_start(out=xt[:, :, :], in_=pred3[gsl, k, :, :])
                    nc.vector.reduce_max(
                        out=M1[:, g, k * CSUB : (k + 1) * CSUB],
                        in_=xt[:, :, :],
                        axis=mybir.AxisListType.X,
                    )
                phase_a(g)
                if g >= 1:
                    phase_b(g - 1)
                if g >= 2:
                    phase_c(g - 2)

              phase_b(G - 1)
              phase_c(G - 2)
              phase_c(G - 1)

              # ---- batched finish ----------------------------------------
              # cosine: dist = dot / (max(sqrt(na2),eps)*max(sqrt(nb2),eps))
              nanb = per.tile([P, 2 * G], f32, name=f"nanb{rep}", tag="nanb")
              nc.scalar.activation(nanb[:, :], nanb2[:, :],
                                   mybir.ActivationFunctionType.Sqrt)
              nc.vector.tensor_scalar_max(nanb[:, :], nanb[:, :], EPS)
              den = per.tile([P, G], f32, name=f"den{rep}", tag="den")
              nc.vector.tensor_mul(den[:, :], nanb[:, 0:G], nanb[:, G : 2 * G])
              rden = per.tile([P, G], f32, name=f"rden{rep}", tag="rden")
              nc.vector.reciprocal(rden[:, :], den[:, :])
              dist = per.tile([P, G], f32, name=f"dist{rep}", tag="dist")
              nc.vector.tensor_mul(dist[:, :], dot[:, :], rden[:, :])

              # sig = sigmoid(alpha*dist + beta)
              sarg = per.tile([P, G], f32, name=f"sarg{rep}", tag="sarg")
              nc.vector.tensor_mul(sarg[:, :], dist[:, :], abg[:, :, 0])
              nc.vector.tensor_add(sarg[:, :], sarg[:, :], abg[:, :, 1])
              sig = per.tile([P, G], f32, name=f"sig{rep}", tag="sig")
              nc.scalar.activation(sig[:, :], sarg[:, :],
                                   mybir.ActivationFunctionType.Sigmoid)

              # loss*mask = (sig/2)*(nllm - plm) + 0.5*(nllm + plm)
              #   nllm = nll*mask = t_val*mn;  plm = -m*mask = m_all*mn
              nllm = per.tile([P, G], f32, name=f"nllm{rep}", tag="nllm")
              nc.vector.tensor_mul(nllm[:, :], t_val[:, :], mn[:, :])
              plm = per.tile([P, G], f32, name=f"plm{rep}", tag="plm")
              nc.vector.tensor_mul(plm[:, :], m_all[:, :], mn[:, :])
              d1 = per.tile([P, G], f32, name=f"d1_{rep}", tag="d1")
              nc.vector.tensor_sub(d1[:, :], nllm[:, :], plm[:, :])
              d2 = per.tile([P, G], f32, name=f"d2_{rep}", tag="d2")
              nc.vector.tensor_add(d2[:, :], nllm[:, :], plm[:, :])
              xd = per.tile([P, G], f32, name=f"xd{rep}", tag="xd")
              nc.vector.tensor_mul(xd[:, :], sig[:, :], d1[:, :])
              lt = per.tile([P, G], f32, name=f"lt{rep}", tag="lt")
              nc.vector.tensor_add(lt[:, :], xd[:, :], d2[:, :])
              loss_t = per.tile([P, G], f32, name=f"loss_t{rep}", tag="loss_t")
              nc.vector.tensor_scalar_mul(loss_t[:, :], lt[:, :], 0.5)

              vals = per.tile([P, 2], f32, name=f"vals{rep}", tag="vals")
              nc.vector.reduce_sum(out=vals[:, 0:1], in_=loss_t[:, :],
                                   axis=mybir.AxisListType.X)
              nc.vector.reduce_sum(out=vals[:, 1:2], in_=nllm[:, :],
                                   axis=mybir.AxisListType.X)
              acc = psum.tile([1, 2], f32, space="PSUM", name=f"acc{rep}",
                              tag="acc")
              nc.tensor.matmul(out=acc[:, :], lhsT=ones[:, :], rhs=vals[:, :],
                               start=True, stop=True)
              res = per.tile([1, 2], f32, name=f"res{rep}", tag="res2")
              nc.vector.tensor_copy(res[:, :], acc[:, :])
              nc.sync.dma_start(out=out[:, :], in_=res[:, :])

    nc.compile()
    _CACHE[key] = nc
    return nc


def _host_constants():
    toks = np.arange(TOK, dtype=np.int64)
    rbase = (toks * NS).astype(np.int32).reshape(G, P).T.copy()
    return rbase


def _in_maps(pred_ll, target, weight, alpha, beta):
    rbase = _host_constants()
    pred_ll = np.ascontiguousarray(pred_ll, dtype=np.float32)
    weight = np.ascontiguousarray(weight, dtype=np.float32)
    ab = np.ascontiguousarray(
        np.stack([np.asarray(alpha, np.float32), np.asarray(beta, np.float32)],
                 axis=1)
    )
    tgt64 = np.asarray(target).astype(np.int64)
    toks = np.arange(TOK, dtype=np.int64)

    in_maps = []
    for c in range(CORES):
        tl = tgt64[c * TOK : (c + 1) * TOK]
        fidx = (toks * V + tl).astype(np.int32)
        in_maps.append({
            "pred": pred_ll[c * TOK : (c + 1) * TOK],
            "tgt": np.ascontiguousarray(tl.astype(np.int32).reshape(G, P).T),
            "fidx": np.ascontiguousarray(fidx.reshape(G, P).T),
            "rbase": rbase,
            "wt": weight,
            "ab": ab,
        })
    return in_maps


def _finish(results):
    partial = np.stack([r["out"].reshape(2) for r in results])  # [8, 2]
    loss_sum, nll_sum = np.asarray(partial, np.float64).sum(axis=0)
    return (np.float32(loss_sum), np.float32(nll_sum))


def kernel(pred_ll, target, weight, alpha, beta):
    nc = _build()
    in_maps = _in_maps(pred_ll, target, weight, alpha, beta)
    res = bass_utils.run_bass_kernel_spmd(nc, in_maps, core_ids=list(range(CORES)))
    return _finish(res.results)
